# revision 27
# baseline (speedup 1.0000x reference)
"""Multi-head attention (B=4, S=2048, D=1024, H=16) on 8 trn2 NeuronCores.

Sharding: tensor-parallel over heads — core c owns heads (2c, 2c+1).

v2 schedule: the attention inner loop is PE-paced. Projection work is
chopped into small tasks (one 8-matmul half = ~1.7us of PE) interleaved
between attention key-tiles so the tensor engine always has more work per
key-tile (~1.17us) than the scalar engine's exp (~1.03us) — the PE never
stalls on exp and stays at max p-state. PV matmuls lag scores by one
key-tile for the same reason.

Per core:
  1. Q^T/K^T projections into [head_dim-stacked, tokens] bf16; V^T
     projection, then DMA-XBAR transposes into V-natural [tokens, head_dim]
     with fused ones-columns (softmax row sums accumulate on PSUM row 64).
  2. Attention per (batch, q-window): scores^T = K^T.T @ Q^T (2-head
     row-packed matmuls), exp on ScalarE (PSUM -> SBUF bf16), PV with
     lhsT=[V|1]. Normalize via single-lane reciprocal + gpsimd
     partition_broadcast (no DRAM bounce).
  3. AllToAll (heads-sharded -> token-sharded), then the output projection
     for this core's 1024-token slice.

Host side folds the 1/sqrt(head_dim) scale into w_q, pre-transposes and
pre-tiles all operands to bf16, and assembles the [4,2048,1024] fp32 output.

Biases are applied exactly on the host: b_v and b_o contribute
(b_v @ w_o.T + b_o) to every token (softmax rows sum to 1). b_q/b_k cannot
be folded; setup_inputs() generates them as zeros — a numpy fallback guards
the (never-exercised) nonzero case, as well as non-trivial masks.
"""

import numpy as np
import ml_dtypes

import concourse.bass as bass
import concourse.tile as tile
from concourse import mybir
from concourse.bass_utils import run_bass_kernel_spmd
from concourse.masks import make_identity

NCORES = 8
B, S, D, H = 4, 2048, 1024, 16
HD = D // H            # 64
P = 128
T = B * S              # 8192 tokens
TOK_PER_CORE = T // NCORES   # 1024
NCH = D // P           # 8 contraction chunks
NT2 = T // 1024        # 8 token tiles of 1024 for projections
NKT = S // P           # 16 key tiles per batch
NQW = S // 512         # 4 q-windows of 512 per batch
VROW = 2 * (HD + 1)    # 130 cols per k-tile in v_all ([V_h0|1|V_h1|1])

BF16 = mybir.dt.bfloat16
F32 = mybir.dt.float32
bf16 = ml_dtypes.bfloat16

_CACHED_NC = None


def split_multi_waits(nc):
    """This walrus build supports one sync-wait per instruction; hoist extras
    onto same-engine NoOps inserted immediately before."""
    for f in nc.m.functions:
        for blk in f.blocks:
            insts = blk.instructions
            i = 0
            while i < len(insts):
                inst = insts[i]
                si = getattr(inst, "sync_info", None)
                if si is not None and si.on_wait and len(si.on_wait) > 1:
                    waits = list(si.on_wait)
                    for j, w in enumerate(waits[:-1]):
                        nop = mybir.InstNoOp(name=f"I-ws-{inst.name}-{j}",
                                             ins=[], outs=[])
                        nop.engine = inst.engine
                        nop.sync_info = mybir.SyncInfo(on_wait=[w], on_update=[])
                        insts.insert(i, nop)
                        i += 1
                    inst.sync_info = mybir.SyncInfo(on_wait=[waits[-1]],
                                                    on_update=si.on_update)
                i += 1


def build(split=True):
    global _CACHED_NC
    if split and _CACHED_NC is not None:
        return _CACHED_NC
    from contextlib import ExitStack

    nc = bass.Bass(num_devices=NCORES, target_bir_lowering=False, debug=False)

    # Inputs (per core). x* are the full activations, tiled on host to
    # [toktile, chunk, 128, 1024].
    xq_d = nc.dram_tensor("xq", [NT2, NCH, P, 1024], BF16, kind="ExternalInput")
    xk_d = nc.dram_tensor("xk", [NT2, NCH, P, 1024], BF16, kind="ExternalInput")
    xv_d = nc.dram_tensor("xv", [NT2, NCH, P, 1024], BF16, kind="ExternalInput")
    wq_d = nc.dram_tensor("wq", [NCH, P, P], BF16, kind="ExternalInput")
    wk_d = nc.dram_tensor("wk", [NCH, P, P], BF16, kind="ExternalInput")
    wv_d = nc.dram_tensor("wv", [NCH, P, P], BF16, kind="ExternalInput")
    wo_d = nc.dram_tensor("wo", [NCH, P, 1024], BF16, kind="ExternalInput")
    out_d = nc.dram_tensor("out", [TOK_PER_CORE, D], F32, kind="ExternalOutput")

    # Two half-token AllToAll stages (buf0 = dest cols 0:512, buf1 = 512:1024).
    a2a_in = [nc.dram_tensor(f"a2a_in{h}", [NCORES, P, 512], BF16) for h in (0, 1)]
    a2a_out = [nc.dram_tensor(f"a2a_out{h}", [NCORES, P, 512], BF16) for h in (0, 1)]
    # rowsum/reciprocal bounces for the partition-broadcast DMA
    rs_d = nc.dram_tensor("rs_d", [B * NQW, 1024], F32)
    rcp_d = nc.dram_tensor("rcp_d", [B * NQW, 1024], F32)

    with tile.TileContext(nc, pool_alloc_mode="queue") as tc:
        with ExitStack() as ctx:
            const = ctx.enter_context(tc.tile_pool(name="const", bufs=1))
            persist = ctx.enter_context(tc.tile_pool(name="persist", bufs=1))
            xin = ctx.enter_context(tc.tile_pool(name="xin", bufs=3))
            work = ctx.enter_context(tc.tile_pool(name="work", bufs=2))
            expool = ctx.enter_context(tc.tile_pool(name="expool", bufs=4))
            npool = ctx.enter_context(tc.tile_pool(name="npool", bufs=2))
            psum = ctx.enter_context(tc.tile_pool(name="psum", bufs=2, space="PSUM"))

            ident = const.tile([P, P], BF16)
            make_identity(nc, ident)
            # ones row-vector at partition 64 (lhsT of the PE recip-broadcast)
            ones64 = const.tile([HD + 1, HD], BF16)
            nc.vector.memset(ones64[HD:HD + 1, :], 1.0)

            # Persistent SBUF: Qt/Kt [hd2, tokens], V-natural-with-ones, w_o.
            qt_sb = persist.tile([P, T], BF16, tag="qt_sb")
            kt_sb = persist.tile([P, T], BF16, tag="kt_sb")
            v_all = persist.tile([P, B * NKT * VROW], BF16, tag="v_all")
            wo_sb = persist.tile([P, NCH * 1024], BF16, tag="wo_sb")
            wq_sb = persist.tile([P, NCH * P], BF16, tag="wq_sb")
            wk_sb = persist.tile([P, NCH * P], BF16, tag="wk_sb")
            wv_sb = persist.tile([P, NCH * P], BF16, tag="wv_sb")

            # ones columns of v_all (cols 64 and 129 of each 130-block)
            v_view = v_all[:].rearrange("p (n c) -> p n c", c=VROW)
            nc.vector.memset(v_view[:, :, HD], 1.0)
            nc.vector.memset(v_view[:, :, 2 * HD + 1], 1.0)

            # weights: per-chunk DMAs on queues that are idle at start
            for ch in range(NCH):
                nc.scalar.dma_start(wk_sb[:, ch * P:(ch + 1) * P], wk_d.ap()[ch])
                nc.gpsimd.dma_start(wv_sb[:, ch * P:(ch + 1) * P], wv_d.ap()[ch])
                nc.scalar.dma_start(wq_sb[:, ch * P:(ch + 1) * P], wq_d.ap()[ch])

            # ---- projection task emitters ----
            xt_tiles = {}
            pend_vt = {}
            x_ds = {"q": xq_d, "k": xk_d, "v": xv_d}
            w_sbs = {"q": wq_sb, "k": wk_sb, "v": wv_sb}

            def dma_task(which, t2):
                def run():
                    t = xin.tile([P, NCH * 1024], BF16, tag="xt")
                    xt_tiles[(which, t2)] = t
                    for ch in range(NCH):
                        nc.sync.dma_start(t[:, ch * 1024:(ch + 1) * 1024],
                                          x_ds[which].ap()[t2, ch])
                return run

            def half_task(which, t2, half):
                def run():
                    xt = xt_tiles[(which, t2)]
                    w_sb = w_sbs[which]
                    ps = psum.tile([P, 1024], F32, tag="sc")
                    for ch in range(NCH):
                        nc.tensor.matmul(
                            ps[:, 0:512], w_sb[:, ch * P:(ch + 1) * P],
                            xt[:, ch * 1024 + half * 512: ch * 1024 + (half + 1) * 512],
                            start=(ch == 0), stop=(ch == NCH - 1))
                    col = t2 * 1024 + half * 512
                    if which == "q":
                        nc.vector.tensor_copy(qt_sb[:, col:col + 512], ps[:, 0:512])
                    elif which == "k":
                        nc.vector.tensor_copy(kt_sb[:, col:col + 512], ps[:, 0:512])
                    else:
                        vt = work.tile([P, 512], BF16, tag="vt")
                        nc.vector.tensor_copy(vt[:], ps[:, 0:512])
                        pend_vt[(t2, half)] = vt
                return run

            def vtrans_task(t2, half):
                # PE transpose V^T [dims, tokens] -> V-natural blocks in
                # v_all, writing around the ones columns. Scratch PSUM comes
                # from the sc rotation (held ~1.5us only).
                def run():
                    vt = pend_vt.pop((t2, half))
                    col = t2 * 1024 + half * 512
                    ps = psum.tile([P, 1024], F32, tag="sc")
                    tp4 = ps[:, 512:768].bitcast(BF16)
                    for j in range(4):
                        tp = tp4[:, j * P:(j + 1) * P]
                        nc.tensor.transpose(tp[:], vt[:, j * P:(j + 1) * P],
                                            ident[:])
                        g = col + j * P
                        bb, ktb = g // S, (g % S) // P
                        base = (bb * NKT + ktb) * VROW
                        nc.vector.tensor_copy(
                            v_all[:, base:base + VROW]
                            .rearrange("p (h c) -> p h c", c=HD + 1)[:, :, 0:HD],
                            tp[:].rearrange("p (h c) -> p h c", c=HD))
                return run

            def wo_task():
                for ch in range(NCH):
                    nc.gpsimd.dma_start(wo_sb[:, ch * 1024:(ch + 1) * 1024],
                                        wo_d.ap()[ch])

            def batch_tasks(b):
                a, b2 = 2 * b, 2 * b + 1
                return [
                    dma_task("k", a), dma_task("v", a), dma_task("k", b2),
                    half_task("k", a, 0), half_task("k", a, 1), dma_task("v", b2),
                    half_task("k", b2, 0), half_task("k", b2, 1),
                    half_task("v", a, 0), half_task("v", a, 1), dma_task("q", a),
                    vtrans_task(a, 0), vtrans_task(a, 1),
                    half_task("v", b2, 0), half_task("v", b2, 1), dma_task("q", b2),
                    vtrans_task(b2, 0), vtrans_task(b2, 1),
                    half_task("q", a, 0), half_task("q", a, 1),
                    half_task("q", b2, 0), half_task("q", b2, 1),
                ]

            # ---- attention ----
            def emit_pv(b, kt, ex, pv0, pv1):
                vb = (b * NKT + kt) * VROW
                nc.tensor.matmul(
                    pv0[:], v_all[:, vb:vb + HD + 1], ex[:, 0:512],
                    start=(kt == 0), stop=(kt == NKT - 1))
                nc.tensor.matmul(
                    pv1[:], v_all[:, vb + HD + 1:vb + VROW], ex[:, 512:1024],
                    start=(kt == 0), stop=(kt == NKT - 1))

            pending_norm = [None]

            def flush_norm():
                if pending_norm[0] is not None:
                    pending_norm[0]()
                    pending_norm[0] = None

            def attn_unit(b, qw, fill, fstate):
                buf = qw % 2
                dest = 2 * b + qw // 2
                qcol = b * S + qw * 512
                pv0 = psum.tile([HD + 1, 512], F32, tag="pv0")
                pv1 = psum.tile([HD + 1, 512], F32, tag="pv1")
                exq = []
                for kt in range(NKT):
                    kcol = b * S + kt * P
                    sc = psum.tile([P, 1024], F32, tag="sc")
                    nc.tensor.matmul(
                        sc[:, 0:512],
                        kt_sb[0:HD, kcol:kcol + P],
                        qt_sb[0:HD, qcol:qcol + 512],
                        start=True, stop=True, tile_position=(0, 0))
                    nc.tensor.matmul(
                        sc[:, 512:1024],
                        kt_sb[HD:2 * HD, kcol:kcol + P],
                        qt_sb[HD:2 * HD, qcol:qcol + 512],
                        start=True, stop=True, tile_position=(HD, 0))
                    ex = expool.tile([P, 1024], BF16, tag="ex")
                    nc.scalar.activation(
                        ex[:], sc[:], mybir.ActivationFunctionType.Exp)
                    exq.append((kt, ex))
                    if kt == 10:
                        # previous unit's broadcast+scale: its ~9us DVE recip
                        # chain is done by now, so the PE won't stall on it
                        flush_norm()
                    # filler: keep the PE ahead of the scalar engine
                    fstate[0] += 1
                    target = (len(fill) * fstate[0] + fstate[1] - 1) // fstate[1]
                    while fstate[2] < min(target, len(fill)):
                        fill[fstate[2]]()
                        fstate[2] += 1
                    if len(exq) > 1:
                        k0, e0 = exq.pop(0)
                        emit_pv(b, k0, e0, pv0, pv1)
                for k0, e0 in exq:
                    emit_pv(b, k0, e0, pv0, pv1)
                # normalize, fully in SBUF/PSUM (no DRAM bounce): copy pv off
                # PSUM (frees the accumulators), approx-reciprocal the
                # ones-rows; then (deferred into the next unit) broadcast the
                # recips across partitions with K=1 PE matmuls
                # (ones64 x recip-row), scale, and write the a2a input.
                pva = npool.tile([HD + 1, 512], F32, tag="pva")
                pvb = npool.tile([HD + 1, 512], F32, tag="pvb")
                nc.vector.tensor_copy(pva[:], pv0[:])
                nc.vector.tensor_copy(pvb[:], pv1[:])
                rc = npool.tile([HD + 1, 1024], F32, tag="rc")
                nc.vector.reciprocal(rc[HD:HD + 1, 0:512], pva[HD:HD + 1, :])
                nc.vector.reciprocal(rc[HD:HD + 1, 512:1024], pvb[HD:HD + 1, :])
                rcb = npool.tile([HD + 1, 1024], BF16, tag="rcb")
                nc.vector.tensor_copy(rcb[HD:HD + 1, :], rc[HD:HD + 1, :])

                def norm_post():
                    bca = psum.tile([HD, 512], F32, tag="pv0")
                    bcb = psum.tile([HD, 512], F32, tag="pv1")
                    nc.tensor.matmul(bca[:], ones64[HD:HD + 1, :],
                                     rcb[HD:HD + 1, 0:512],
                                     start=True, stop=True)
                    nc.tensor.matmul(bcb[:], ones64[HD:HD + 1, :],
                                     rcb[HD:HD + 1, 512:1024],
                                     start=True, stop=True)
                    at = npool.tile([P, 512], BF16, tag="at")
                    nc.vector.tensor_mul(at[0:HD, :], pva[0:HD, :], bca[:])
                    nc.vector.tensor_mul(at[HD:P, :], pvb[0:HD, :], bcb[:])
                    nc.gpsimd.dma_start(a2a_in[buf].ap()[dest], at[:])

                pending_norm[0] = norm_post

            def collective(buf):
                nc.gpsimd.collective_compute(
                    "AllToAll", mybir.AluOpType.bypass,
                    replica_groups=[list(range(NCORES))],
                    ins=[a2a_in[buf].ap()], outs=[a2a_out[buf].ap()],
                )

            # ---- output projection for one half of our token slice ----
            def outproj(buf):
                gsb = persist.tile([P, NCH * 512], BF16, tag=f"gsb{buf}")
                for ch in range(NCH):
                    nc.sync.dma_start(gsb[:, ch * 512:(ch + 1) * 512],
                                      a2a_out[buf].ap()[ch])
                for t128 in range(4):
                    for dhalf in range(2):
                        po = psum.tile([P, 512], F32, tag=("pv0", "pv1")[dhalf])
                        for ch in range(NCH):
                            nc.tensor.matmul(
                                po[:, 0:512],
                                gsb[:, ch * 512 + t128 * P: ch * 512 + (t128 + 1) * P],
                                wo_sb[:, ch * 1024 + dhalf * 512: ch * 1024 + (dhalf + 1) * 512],
                                start=(ch == 0), stop=(ch == NCH - 1))
                        osb = work.tile([P, 512], F32, tag="osb")
                        nc.vector.tensor_copy(osb[:], po[:])
                        row = buf * 512 + t128 * P
                        nc.sync.dma_start(
                            out_d.ap()[row:row + P, dhalf * 512:(dhalf + 1) * 512],
                            osb[:])

            # ---- schedule ----
            # prologue: batch 0's K fully, V(t2=0) + its vtranses, and
            # Q(t2=0, half 0) — just enough for qw0. The second V token-tile,
            # remaining q-halves, and their vtranses become early fillers
            # (v blocks 8-15 aren't consumed until kt8+ of the first unit).
            t0 = batch_tasks(0)
            for i in list(range(13)) + [18]:
                t0[i]()
            fill_b0_extra = [t0[i] for i in (15, 13, 14, 16, 17, 19, 20, 21)]

            # b3's attention has no next batch to project; defer a few of its
            # own late-needed tasks (vtranses for its second token tile + 3
            # q-halves) into b3's first unit so the PE stays ahead of exp.
            # b3 runs qw order (1,3,0,2): the cc1 contributors finish two
            # units early so the AllToAlls overlap compute and outproj.
            t3 = batch_tasks(3)
            defer = (16, 17, 21, 18, 20)
            fills = [
                fill_b0_extra + batch_tasks(1),
                batch_tasks(2) + [wo_task],
                [t for i, t in enumerate(t3) if i not in defer],
                [t3[i] for i in defer],
            ]
            for b in range(B):
                fill = fills[b]
                # b3's deferred fillers must all land in its first unit
                fstate = [0, NKT if b == B - 1 else NQW * NKT, 0]
                qws = (1, 3, 0, 2) if b == B - 1 else (0, 2, 1, 3)
                for qw in qws:
                    attn_unit(b, qw, fill, fstate)
                    if b == B - 1 and qw == 0:
                        # (3,1) and (3,3) norms have flushed by now
                        collective(1)
                while fstate[2] < len(fill):
                    fill[fstate[2]]()
                    fstate[2] += 1
            # PE: outproj(1) runs while (3,2)'s recip chain drains and cc0
            # flies; outproj(0) follows once cc0 lands.
            outproj(1)
            flush_norm()
            collective(0)
            outproj(0)

    if split:
        split_multi_waits(nc)
        _CACHED_NC = nc
    return nc


def _host_prep(query, key, value, w_q, w_k, w_v, w_o):
    sc = 1.0 / np.sqrt(np.float32(HD))

    def tile_x(x):  # [B,S,D] -> [NT2, NCH, 128, 1024] bf16 of x^T
        xt = np.asarray(x, np.float32).reshape(T, D).T          # [D, T]
        xt = xt.reshape(NCH, P, NT2, 1024).transpose(2, 0, 1, 3)
        return np.ascontiguousarray(xt.astype(bf16))

    xq, xk, xv = tile_x(query), tile_x(key), tile_x(value)

    def tile_w(w, c, scale=1.0):  # rows for core c, transposed, chunked
        wc = (np.asarray(w, np.float32)[P * c:P * (c + 1), :] * scale).T  # [D,128]
        return np.ascontiguousarray(wc.reshape(NCH, P, P).astype(bf16))

    wo_t = np.ascontiguousarray(
        np.asarray(w_o, np.float32).T.reshape(NCH, P, 1024).astype(bf16))

    in_maps = []
    for c in range(NCORES):
        in_maps.append({
            "xq": xq, "xk": xk, "xv": xv,
            "wq": tile_w(w_q, c, sc), "wk": tile_w(w_k, c),
            "wv": tile_w(w_v, c), "wo": wo_t,
        })
    return in_maps


def _numpy_fallback(query, key, value, attn_mask, key_padding_mask,
                    w_q, b_q, w_k, b_k, w_v, b_v, w_o, b_o):
    q = query.reshape(T, D) @ w_q.T + b_q
    k = key.reshape(T, D) @ w_k.T + b_k
    v = value.reshape(T, D) @ w_v.T + b_v
    qh = q.reshape(B, S, H, HD).transpose(0, 2, 1, 3)
    kh = k.reshape(B, S, H, HD).transpose(0, 2, 1, 3)
    vh = v.reshape(B, S, H, HD).transpose(0, 2, 1, 3)
    out = np.empty((B, H, S, HD), np.float32)
    neg = np.finfo(np.float32).min
    for b in range(B):
        for h in range(H):
            s = (qh[b, h] @ kh[b, h].T) / np.sqrt(np.float32(HD))
            s = np.where(attn_mask, s, neg)
            s = np.where(key_padding_mask[b][None, :], s, neg)
            s = s - s.max(axis=-1, keepdims=True)
            e = np.exp(s)
            a = e / e.sum(axis=-1, keepdims=True)
            out[b, h] = a @ vh[b, h]
    o = out.transpose(0, 2, 1, 3).reshape(T, D)
    return (o @ w_o.T + b_o).reshape(B, S, D).astype(np.float32)


def kernel(query, key, value, attn_mask, key_padding_mask,
           w_q, b_q, w_k, b_k, w_v, b_v, w_o, b_o):
    query = np.asarray(query, np.float32)
    key = np.asarray(key, np.float32)
    value = np.asarray(value, np.float32)
    attn_mask = np.asarray(attn_mask)
    key_padding_mask = np.asarray(key_padding_mask)
    w_q, b_q = np.asarray(w_q, np.float32), np.asarray(b_q, np.float32)
    w_k, b_k = np.asarray(w_k, np.float32), np.asarray(b_k, np.float32)
    w_v, b_v = np.asarray(w_v, np.float32), np.asarray(b_v, np.float32)
    w_o, b_o = np.asarray(w_o, np.float32), np.asarray(b_o, np.float32)

    if (not attn_mask.all() or not key_padding_mask.all()
            or b_q.any() or b_k.any()):
        return _numpy_fallback(query, key, value, attn_mask, key_padding_mask,
                               w_q, b_q, w_k, b_k, w_v, b_v, w_o, b_o)

    nc = build()
    in_maps = _host_prep(query, key, value, w_q, w_k, w_v, w_o)
    res = run_bass_kernel_spmd(nc, in_maps, list(range(NCORES)))

    out = np.empty((T, D), np.float32)
    for c in range(NCORES):
        out[TOK_PER_CORE * c:TOK_PER_CORE * (c + 1)] = \
            res.results[c]["out"].reshape(TOK_PER_CORE, D)
    # exact host-side bias fold: softmax rows sum to 1 => + (b_v @ w_o.T + b_o)
    out += b_v @ w_o.T + b_o
    return out.reshape(B, S, D)


# revision 32
# speedup vs baseline: 1.1731x; 1.1731x over previous
"""Multi-head attention (B=4, S=2048, D=1024, H=16) on 8 trn2 NeuronCores.

Sharding: tensor-parallel over heads — core c owns heads (2c, 2c+1).

v2 schedule: the attention inner loop is PE-paced. Projection work is
chopped into small tasks (one 8-matmul half = ~1.7us of PE) interleaved
between attention key-tiles so the tensor engine always has more work per
key-tile (~1.17us) than the scalar engine's exp (~1.03us) — the PE never
stalls on exp and stays at max p-state. PV matmuls lag scores by one
key-tile for the same reason.

Per core:
  1. Q^T/K^T projections into [head_dim-stacked, tokens] bf16; V^T
     projection, then DMA-XBAR transposes into V-natural [tokens, head_dim]
     with fused ones-columns (softmax row sums accumulate on PSUM row 64).
  2. Attention per (batch, q-window): scores^T = K^T.T @ Q^T (2-head
     row-packed matmuls), exp on ScalarE (PSUM -> SBUF bf16), PV with
     lhsT=[V|1]. Normalize via single-lane reciprocal + gpsimd
     partition_broadcast (no DRAM bounce).
  3. AllToAll (heads-sharded -> token-sharded), then the output projection
     for this core's 1024-token slice.

Host side folds the 1/sqrt(head_dim) scale into w_q, pre-transposes and
pre-tiles all operands to bf16, and assembles the [4,2048,1024] fp32 output.

Biases are applied exactly on the host: b_v and b_o contribute
(b_v @ w_o.T + b_o) to every token (softmax rows sum to 1). b_q/b_k cannot
be folded; setup_inputs() generates them as zeros — a numpy fallback guards
the (never-exercised) nonzero case, as well as non-trivial masks.
"""

import numpy as np
import ml_dtypes

import concourse.bass as bass
import concourse.tile as tile
from concourse import mybir
from concourse.bass_utils import run_bass_kernel_spmd
from concourse.masks import make_identity

NCORES = 8
B, S, D, H = 4, 2048, 1024, 16
HD = D // H            # 64
P = 128
T = B * S              # 8192 tokens
TOK_PER_CORE = T // NCORES   # 1024
NCH = D // P           # 8 contraction chunks
NT2 = T // 1024        # 8 token tiles of 1024 for projections
NKT = S // P           # 16 key tiles per batch
NQW = S // 512         # 4 q-windows of 512 per batch
VROW = 2 * (HD + 1)    # 130 cols per k-tile in v_all ([V_h0|1|V_h1|1])

BF16 = mybir.dt.bfloat16
F32 = mybir.dt.float32
bf16 = ml_dtypes.bfloat16

_CACHED_NC = None


def split_multi_waits(nc):
    """This walrus build supports one sync-wait per instruction; hoist extras
    onto same-engine NoOps inserted immediately before."""
    for f in nc.m.functions:
        for blk in f.blocks:
            insts = blk.instructions
            i = 0
            while i < len(insts):
                inst = insts[i]
                si = getattr(inst, "sync_info", None)
                if si is not None and si.on_wait and len(si.on_wait) > 1:
                    waits = list(si.on_wait)
                    for j, w in enumerate(waits[:-1]):
                        nop = mybir.InstNoOp(name=f"I-ws-{inst.name}-{j}",
                                             ins=[], outs=[])
                        nop.engine = inst.engine
                        nop.sync_info = mybir.SyncInfo(on_wait=[w], on_update=[])
                        insts.insert(i, nop)
                        i += 1
                    inst.sync_info = mybir.SyncInfo(on_wait=[waits[-1]],
                                                    on_update=si.on_update)
                i += 1


def build(split=True):
    global _CACHED_NC
    if split and _CACHED_NC is not None:
        return _CACHED_NC
    from contextlib import ExitStack

    nc = bass.Bass(num_devices=NCORES, target_bir_lowering=False, debug=False)

    # Inputs (per core). x* are the full activations, tiled on host to
    # [toktile, chunk, 128, 1024].
    xq_d = nc.dram_tensor("xq", [NT2, NCH, P, 1024], BF16, kind="ExternalInput")
    xk_d = nc.dram_tensor("xk", [NT2, NCH, P, 1024], BF16, kind="ExternalInput")
    xv_d = nc.dram_tensor("xv", [NT2, NCH, P, 1024], BF16, kind="ExternalInput")
    wq_d = nc.dram_tensor("wq", [NCH, P, P], BF16, kind="ExternalInput")
    wk_d = nc.dram_tensor("wk", [NCH, P, P], BF16, kind="ExternalInput")
    wv_d = nc.dram_tensor("wv", [NCH, P, P], BF16, kind="ExternalInput")
    wo_d = nc.dram_tensor("wo", [NCH, P, 1024], BF16, kind="ExternalInput")
    out_d = nc.dram_tensor("out", [TOK_PER_CORE, D], F32, kind="ExternalOutput")

    # Two half-token AllToAll stages (buf0 = dest cols 0:512, buf1 = 512:1024).
    a2a_in = [nc.dram_tensor(f"a2a_in{h}", [NCORES, P, 512], BF16) for h in (0, 1)]
    a2a_out = [nc.dram_tensor(f"a2a_out{h}", [NCORES, P, 512], BF16) for h in (0, 1)]
    # rowsum/reciprocal bounces for the partition-broadcast DMA
    rs_d = nc.dram_tensor("rs_d", [B * NQW, 1024], F32)
    rcp_d = nc.dram_tensor("rcp_d", [B * NQW, 1024], F32)

    with tile.TileContext(nc, pool_alloc_mode="queue") as tc:
        with ExitStack() as ctx:
            const = ctx.enter_context(tc.tile_pool(name="const", bufs=1))
            persist = ctx.enter_context(tc.tile_pool(name="persist", bufs=1))
            xin = ctx.enter_context(tc.tile_pool(name="xin", bufs=3))
            work = ctx.enter_context(tc.tile_pool(name="work", bufs=2))
            expool = ctx.enter_context(tc.tile_pool(name="expool", bufs=4))
            npool = ctx.enter_context(tc.tile_pool(name="npool", bufs=2))
            psum = ctx.enter_context(tc.tile_pool(name="psum", bufs=2, space="PSUM"))

            ident = const.tile([P, P], BF16)
            make_identity(nc, ident)

            # Persistent SBUF: Qt/Kt [hd2, tokens], V-natural-with-ones, w_o.
            qt_sb = persist.tile([P, T], BF16, tag="qt_sb")
            kt_sb = persist.tile([P, T], BF16, tag="kt_sb")
            v_all = persist.tile([P, B * NKT * VROW], BF16, tag="v_all")
            wo_sb = persist.tile([P, NCH * 1024], BF16, tag="wo_sb")
            wq_sb = persist.tile([P, NCH * P], BF16, tag="wq_sb")
            wk_sb = persist.tile([P, NCH * P], BF16, tag="wk_sb")
            wv_sb = persist.tile([P, NCH * P], BF16, tag="wv_sb")

            # ones columns of v_all (cols 64 and 129 of each 130-block)
            v_view = v_all[:].rearrange("p (n c) -> p n c", c=VROW)
            nc.vector.memset(v_view[:, :, HD], 1.0)
            nc.vector.memset(v_view[:, :, 2 * HD + 1], 1.0)

            # weights: per-chunk DMAs on queues that are idle at start
            for ch in range(NCH):
                nc.scalar.dma_start(wk_sb[:, ch * P:(ch + 1) * P], wk_d.ap()[ch])
                nc.gpsimd.dma_start(wv_sb[:, ch * P:(ch + 1) * P], wv_d.ap()[ch])
                nc.scalar.dma_start(wq_sb[:, ch * P:(ch + 1) * P], wq_d.ap()[ch])

            # ---- projection task emitters ----
            xt_tiles = {}
            pend_vt = {}
            x_ds = {"q": xq_d, "k": xk_d, "v": xv_d}
            w_sbs = {"q": wq_sb, "k": wk_sb, "v": wv_sb}

            def dma_task(which, t2):
                def run():
                    t = xin.tile([P, NCH * 1024], BF16, tag="xt")
                    xt_tiles[(which, t2)] = t
                    for ch in range(NCH):
                        nc.sync.dma_start(t[:, ch * 1024:(ch + 1) * 1024],
                                          x_ds[which].ap()[t2, ch])
                return run

            def half_task(which, t2, half):
                def run():
                    xt = xt_tiles[(which, t2)]
                    w_sb = w_sbs[which]
                    ps = psum.tile([P, 1024], F32, tag="sc")
                    for ch in range(NCH):
                        nc.tensor.matmul(
                            ps[:, 0:512], w_sb[:, ch * P:(ch + 1) * P],
                            xt[:, ch * 1024 + half * 512: ch * 1024 + (half + 1) * 512],
                            start=(ch == 0), stop=(ch == NCH - 1))
                    col = t2 * 1024 + half * 512
                    if which == "q":
                        nc.vector.tensor_copy(qt_sb[:, col:col + 512], ps[:, 0:512])
                    elif which == "k":
                        nc.vector.tensor_copy(kt_sb[:, col:col + 512], ps[:, 0:512])
                    else:
                        vt = work.tile([P, 512], BF16, tag="vt")
                        nc.vector.tensor_copy(vt[:], ps[:, 0:512])
                        pend_vt[(t2, half)] = vt
                return run

            def vtrans_task(t2, half):
                # PE transpose V^T [dims, tokens] -> V-natural blocks in
                # v_all, writing around the ones columns. Scratch PSUM comes
                # from the sc rotation (held ~1.5us only).
                def run():
                    vt = pend_vt.pop((t2, half))
                    col = t2 * 1024 + half * 512
                    ps = psum.tile([P, 1024], F32, tag="sc")
                    tp4 = ps[:, 512:768].bitcast(BF16)
                    for j in range(4):
                        tp = tp4[:, j * P:(j + 1) * P]
                        nc.tensor.transpose(tp[:], vt[:, j * P:(j + 1) * P],
                                            ident[:])
                        g = col + j * P
                        bb, ktb = g // S, (g % S) // P
                        base = (bb * NKT + ktb) * VROW
                        nc.vector.tensor_copy(
                            v_all[:, base:base + VROW]
                            .rearrange("p (h c) -> p h c", c=HD + 1)[:, :, 0:HD],
                            tp[:].rearrange("p (h c) -> p h c", c=HD))
                return run

            def wo_task():
                for ch in range(NCH):
                    nc.gpsimd.dma_start(wo_sb[:, ch * 1024:(ch + 1) * 1024],
                                        wo_d.ap()[ch])

            def batch_tasks(b):
                a, b2 = 2 * b, 2 * b + 1
                return [
                    dma_task("k", a), dma_task("v", a), dma_task("k", b2),
                    half_task("k", a, 0), half_task("k", a, 1), dma_task("v", b2),
                    half_task("k", b2, 0), half_task("k", b2, 1),
                    half_task("v", a, 0), half_task("v", a, 1), dma_task("q", a),
                    vtrans_task(a, 0), vtrans_task(a, 1),
                    half_task("v", b2, 0), half_task("v", b2, 1), dma_task("q", b2),
                    vtrans_task(b2, 0), vtrans_task(b2, 1),
                    half_task("q", a, 0), half_task("q", a, 1),
                    half_task("q", b2, 0), half_task("q", b2, 1),
                ]

            # ---- attention ----
            def emit_pv(b, kt, ex, pv0, pv1):
                vb = (b * NKT + kt) * VROW
                nc.tensor.matmul(
                    pv0[:], v_all[:, vb:vb + HD + 1], ex[:, 0:512],
                    start=(kt == 0), stop=(kt == NKT - 1))
                nc.tensor.matmul(
                    pv1[:], v_all[:, vb + HD + 1:vb + VROW], ex[:, 512:1024],
                    start=(kt == 0), stop=(kt == NKT - 1))

            def attn_unit(b, qw, fill, fstate):
                buf = qw % 2
                dest = 2 * b + qw // 2
                qcol = b * S + qw * 512
                pv0 = psum.tile([HD + 1, 512], F32, tag="pv0")
                pv1 = psum.tile([HD + 1, 512], F32, tag="pv1")
                exq = []
                for kt in range(NKT):
                    kcol = b * S + kt * P
                    sc = psum.tile([P, 1024], F32, tag="sc")
                    nc.tensor.matmul(
                        sc[:, 0:512],
                        kt_sb[0:HD, kcol:kcol + P],
                        qt_sb[0:HD, qcol:qcol + 512],
                        start=True, stop=True, tile_position=(0, 0))
                    nc.tensor.matmul(
                        sc[:, 512:1024],
                        kt_sb[HD:2 * HD, kcol:kcol + P],
                        qt_sb[HD:2 * HD, qcol:qcol + 512],
                        start=True, stop=True, tile_position=(HD, 0))
                    ex = expool.tile([P, 1024], BF16, tag="ex")
                    nc.scalar.activation(
                        ex[:], sc[:], mybir.ActivationFunctionType.Exp)
                    exq.append((kt, ex))
                    # filler: keep the PE ahead of the scalar engine
                    fstate[0] += 1
                    target = (len(fill) * fstate[0] + fstate[1] - 1) // fstate[1]
                    while fstate[2] < min(target, len(fill)):
                        fill[fstate[2]]()
                        fstate[2] += 1
                    if len(exq) > 1:
                        k0, e0 = exq.pop(0)
                        emit_pv(b, k0, e0, pv0, pv1)
                for k0, e0 in exq:
                    emit_pv(b, k0, e0, pv0, pv1)
                # normalize. Copy pv off PSUM first (frees the accumulators
                # fast); then the reciprocal bounce runs entirely off-SBUF:
                # ones-rows -> DRAM -> [128,8] reshape, cheap recip, -> DRAM
                # -> partition-broadcast DMAs. Triggers split across the
                # gpsimd and sync queues so per-unit waits don't pile up on
                # one queue.
                unit = b * NQW + qw
                pva = npool.tile([HD + 1, 512], F32, tag="pva")
                pvb = npool.tile([HD + 1, 512], F32, tag="pvb")
                nc.vector.tensor_copy(pva[:], pv0[:])
                nc.vector.tensor_copy(pvb[:], pv1[:])
                nc.gpsimd.dma_start(
                    rs_d.ap()[unit, 0:512].rearrange("(a f) -> a f", a=1),
                    pva[HD:HD + 1, :])
                nc.gpsimd.dma_start(
                    rs_d.ap()[unit, 512:1024].rearrange("(a f) -> a f", a=1),
                    pvb[HD:HD + 1, :])
                rsw = npool.tile([P, 8], F32, tag="rsw")
                nc.gpsimd.dma_start(
                    rsw[:], rs_d.ap()[unit].rearrange("(p f) -> p f", f=8))
                rcw = npool.tile([P, 8], F32, tag="rcw")
                nc.vector.reciprocal(rcw[:], rsw[:])
                nc.sync.dma_start(
                    rcp_d.ap()[unit].rearrange("(p f) -> p f", f=8), rcw[:])
                bca = npool.tile([HD, 512], F32, tag="bca")
                bcb = npool.tile([HD, 512], F32, tag="bcb")
                rl = rcp_d.ap()[unit].rearrange("(a f) -> a f", a=1)
                nc.sync.dma_start(bca[:], rl[:, 0:512].to_broadcast((HD, 512)))
                nc.sync.dma_start(bcb[:], rl[:, 512:1024].to_broadcast((HD, 512)))
                at = npool.tile([P, 512], BF16, tag="at")
                nc.vector.tensor_mul(at[0:HD, :], pva[0:HD, :], bca[:])
                nc.vector.tensor_mul(at[HD:P, :], pvb[0:HD, :], bcb[:])
                nc.gpsimd.dma_start(a2a_in[buf].ap()[dest], at[:])

            def collective(buf):
                nc.gpsimd.collective_compute(
                    "AllToAll", mybir.AluOpType.bypass,
                    replica_groups=[list(range(NCORES))],
                    ins=[a2a_in[buf].ap()], outs=[a2a_out[buf].ap()],
                )

            # ---- output projection for one half of our token slice ----
            def outproj(buf):
                gsb = persist.tile([P, NCH * 512], BF16, tag=f"gsb{buf}")
                for ch in range(NCH):
                    nc.sync.dma_start(gsb[:, ch * 512:(ch + 1) * 512],
                                      a2a_out[buf].ap()[ch])
                for t128 in range(4):
                    for dhalf in range(2):
                        po = psum.tile([P, 512], F32, tag=("pv0", "pv1")[dhalf])
                        for ch in range(NCH):
                            nc.tensor.matmul(
                                po[:, 0:512],
                                gsb[:, ch * 512 + t128 * P: ch * 512 + (t128 + 1) * P],
                                wo_sb[:, ch * 1024 + dhalf * 512: ch * 1024 + (dhalf + 1) * 512],
                                start=(ch == 0), stop=(ch == NCH - 1))
                        osb = work.tile([P, 512], F32, tag="osb")
                        nc.vector.tensor_copy(osb[:], po[:])
                        row = buf * 512 + t128 * P
                        nc.sync.dma_start(
                            out_d.ap()[row:row + P, dhalf * 512:(dhalf + 1) * 512],
                            osb[:])

            # ---- schedule ----
            # prologue: batch 0's K fully, V(t2=0) + its vtranses, and
            # Q(t2=0, half 0) — just enough for qw0. The second V token-tile,
            # remaining q-halves, and their vtranses become early fillers
            # (v blocks 8-15 aren't consumed until kt8+ of the first unit).
            t0 = batch_tasks(0)
            for i in list(range(13)) + [18]:
                t0[i]()
            fill_b0_extra = [t0[i] for i in (15, 13, 14, 16, 17, 19, 20, 21)]

            # b3's attention has no next batch to project; defer a few of its
            # own late-needed tasks (vtranses for its second token tile + 3
            # q-halves) into b3's first unit so the PE stays ahead of exp.
            # b3 runs qw order (1,3,0,2): the cc1 contributors finish two
            # units early so the AllToAlls overlap compute and outproj.
            t3 = batch_tasks(3)
            defer = (16, 17, 21, 18, 20)
            fills = [
                fill_b0_extra + batch_tasks(1),
                batch_tasks(2) + [wo_task],
                [t for i, t in enumerate(t3) if i not in defer],
                [t3[i] for i in defer],
            ]
            for b in range(B):
                fill = fills[b]
                # b3's deferred fillers must all land in its first unit
                fstate = [0, NKT if b == B - 1 else NQW * NKT, 0]
                qws = (1, 3, 0, 2) if b == B - 1 else (0, 2, 1, 3)
                for qw in qws:
                    attn_unit(b, qw, fill, fstate)
                    if b == B - 1 and qw == 0:
                        collective(1)
                while fstate[2] < len(fill):
                    fill[fstate[2]]()
                    fstate[2] += 1
            # PE: outproj(1) runs while (3,2)'s recip chain drains and cc0
            # flies; outproj(0) follows once cc0 lands.
            outproj(1)
            collective(0)
            outproj(0)

    if split:
        split_multi_waits(nc)
        _CACHED_NC = nc
    return nc


def _host_prep(query, key, value, w_q, w_k, w_v, w_o):
    sc = 1.0 / np.sqrt(np.float32(HD))

    def tile_x(x):  # [B,S,D] -> [NT2, NCH, 128, 1024] bf16 of x^T
        xt = np.asarray(x, np.float32).reshape(T, D).T          # [D, T]
        xt = xt.reshape(NCH, P, NT2, 1024).transpose(2, 0, 1, 3)
        return np.ascontiguousarray(xt.astype(bf16))

    xq, xk, xv = tile_x(query), tile_x(key), tile_x(value)

    def tile_w(w, c, scale=1.0):  # rows for core c, transposed, chunked
        wc = (np.asarray(w, np.float32)[P * c:P * (c + 1), :] * scale).T  # [D,128]
        return np.ascontiguousarray(wc.reshape(NCH, P, P).astype(bf16))

    wo_t = np.ascontiguousarray(
        np.asarray(w_o, np.float32).T.reshape(NCH, P, 1024).astype(bf16))

    in_maps = []
    for c in range(NCORES):
        in_maps.append({
            "xq": xq, "xk": xk, "xv": xv,
            "wq": tile_w(w_q, c, sc), "wk": tile_w(w_k, c),
            "wv": tile_w(w_v, c), "wo": wo_t,
        })
    return in_maps


def _numpy_fallback(query, key, value, attn_mask, key_padding_mask,
                    w_q, b_q, w_k, b_k, w_v, b_v, w_o, b_o):
    q = query.reshape(T, D) @ w_q.T + b_q
    k = key.reshape(T, D) @ w_k.T + b_k
    v = value.reshape(T, D) @ w_v.T + b_v
    qh = q.reshape(B, S, H, HD).transpose(0, 2, 1, 3)
    kh = k.reshape(B, S, H, HD).transpose(0, 2, 1, 3)
    vh = v.reshape(B, S, H, HD).transpose(0, 2, 1, 3)
    out = np.empty((B, H, S, HD), np.float32)
    neg = np.finfo(np.float32).min
    for b in range(B):
        for h in range(H):
            s = (qh[b, h] @ kh[b, h].T) / np.sqrt(np.float32(HD))
            s = np.where(attn_mask, s, neg)
            s = np.where(key_padding_mask[b][None, :], s, neg)
            s = s - s.max(axis=-1, keepdims=True)
            e = np.exp(s)
            a = e / e.sum(axis=-1, keepdims=True)
            out[b, h] = a @ vh[b, h]
    o = out.transpose(0, 2, 1, 3).reshape(T, D)
    return (o @ w_o.T + b_o).reshape(B, S, D).astype(np.float32)


def kernel(query, key, value, attn_mask, key_padding_mask,
           w_q, b_q, w_k, b_k, w_v, b_v, w_o, b_o):
    query = np.asarray(query, np.float32)
    key = np.asarray(key, np.float32)
    value = np.asarray(value, np.float32)
    attn_mask = np.asarray(attn_mask)
    key_padding_mask = np.asarray(key_padding_mask)
    w_q, b_q = np.asarray(w_q, np.float32), np.asarray(b_q, np.float32)
    w_k, b_k = np.asarray(w_k, np.float32), np.asarray(b_k, np.float32)
    w_v, b_v = np.asarray(w_v, np.float32), np.asarray(b_v, np.float32)
    w_o, b_o = np.asarray(w_o, np.float32), np.asarray(b_o, np.float32)

    if (not attn_mask.all() or not key_padding_mask.all()
            or b_q.any() or b_k.any()):
        return _numpy_fallback(query, key, value, attn_mask, key_padding_mask,
                               w_q, b_q, w_k, b_k, w_v, b_v, w_o, b_o)

    nc = build()
    in_maps = _host_prep(query, key, value, w_q, w_k, w_v, w_o)
    res = run_bass_kernel_spmd(nc, in_maps, list(range(NCORES)))

    out = np.empty((T, D), np.float32)
    for c in range(NCORES):
        out[TOK_PER_CORE * c:TOK_PER_CORE * (c + 1)] = \
            res.results[c]["out"].reshape(TOK_PER_CORE, D)
    # exact host-side bias fold: softmax rows sum to 1 => + (b_v @ w_o.T + b_o)
    out += b_v @ w_o.T + b_o
    return out.reshape(B, S, D)


# revision 39
# speedup vs baseline: 1.2079x; 1.0296x over previous
"""Multi-head attention (B=4, S=2048, D=1024, H=16) on 8 trn2 NeuronCores.

Sharding: tensor-parallel over heads — core c owns heads (2c, 2c+1).

v2 schedule: the attention inner loop is PE-paced. Projection work is
chopped into small tasks (one 8-matmul half = ~1.7us of PE) interleaved
between attention key-tiles so the tensor engine always has more work per
key-tile (~1.17us) than the scalar engine's exp (~1.03us) — the PE never
stalls on exp and stays at max p-state. PV matmuls lag scores by one
key-tile for the same reason.

Per core:
  1. Q^T/K^T projections into [head_dim-stacked, tokens] bf16; V^T
     projection, then DMA-XBAR transposes into V-natural [tokens, head_dim]
     with fused ones-columns (softmax row sums accumulate on PSUM row 64).
  2. Attention per (batch, q-window): scores^T = K^T.T @ Q^T (2-head
     row-packed matmuls), exp on ScalarE (PSUM -> SBUF bf16), PV with
     lhsT=[V|1]. Normalize via single-lane reciprocal + gpsimd
     partition_broadcast (no DRAM bounce).
  3. AllToAll (heads-sharded -> token-sharded), then the output projection
     for this core's 1024-token slice.

Host side folds the 1/sqrt(head_dim) scale into w_q, pre-transposes and
pre-tiles all operands to bf16, and assembles the [4,2048,1024] fp32 output.

Biases are applied exactly on the host: b_v and b_o contribute
(b_v @ w_o.T + b_o) to every token (softmax rows sum to 1). b_q/b_k cannot
be folded; setup_inputs() generates them as zeros — a numpy fallback guards
the (never-exercised) nonzero case, as well as non-trivial masks.
"""

import numpy as np
import ml_dtypes

import concourse.bass as bass
import concourse.tile as tile
from concourse import mybir
from concourse.bass_utils import run_bass_kernel_spmd
from concourse.masks import make_identity

NCORES = 8
B, S, D, H = 4, 2048, 1024, 16
HD = D // H            # 64
P = 128
T = B * S              # 8192 tokens
TOK_PER_CORE = T // NCORES   # 1024
NCH = D // P           # 8 contraction chunks
NT2 = T // 1024        # 8 token tiles of 1024 for projections
NKT = S // P           # 16 key tiles per batch
NQW = S // 512         # 4 q-windows of 512 per batch
VROW = 2 * (HD + 1)    # 130 cols per k-tile in v_all ([V_h0|1|V_h1|1])

BF16 = mybir.dt.bfloat16
F32 = mybir.dt.float32
bf16 = ml_dtypes.bfloat16

_CACHED_NC = None


def split_multi_waits(nc):
    """This walrus build supports one sync-wait per instruction; hoist extras
    onto same-engine NoOps inserted immediately before."""
    for f in nc.m.functions:
        for blk in f.blocks:
            insts = blk.instructions
            i = 0
            while i < len(insts):
                inst = insts[i]
                si = getattr(inst, "sync_info", None)
                if si is not None and si.on_wait and len(si.on_wait) > 1:
                    waits = list(si.on_wait)
                    for j, w in enumerate(waits[:-1]):
                        nop = mybir.InstNoOp(name=f"I-ws-{inst.name}-{j}",
                                             ins=[], outs=[])
                        nop.engine = inst.engine
                        nop.sync_info = mybir.SyncInfo(on_wait=[w], on_update=[])
                        insts.insert(i, nop)
                        i += 1
                    inst.sync_info = mybir.SyncInfo(on_wait=[waits[-1]],
                                                    on_update=si.on_update)
                i += 1


def build(split=True):
    global _CACHED_NC
    if split and _CACHED_NC is not None:
        return _CACHED_NC
    from contextlib import ExitStack

    nc = bass.Bass(num_devices=NCORES, target_bir_lowering=False, debug=False)

    # Inputs (per core). x* are the full activations, tiled on host to
    # [toktile, chunk, 128, 1024].
    xq_d = nc.dram_tensor("xq", [NT2, NCH, P, 1024], BF16, kind="ExternalInput")
    xk_d = nc.dram_tensor("xk", [NT2, NCH, P, 1024], BF16, kind="ExternalInput")
    xv_d = nc.dram_tensor("xv", [NT2, NCH, P, 1024], BF16, kind="ExternalInput")
    wq_d = nc.dram_tensor("wq", [NCH, P, P], BF16, kind="ExternalInput")
    wk_d = nc.dram_tensor("wk", [NCH, P, P], BF16, kind="ExternalInput")
    wv_d = nc.dram_tensor("wv", [NCH, P, P], BF16, kind="ExternalInput")
    wo_d = nc.dram_tensor("wo", [NCH, P, 1024], BF16, kind="ExternalInput")
    out_d = nc.dram_tensor("out", [TOK_PER_CORE, D], F32, kind="ExternalOutput")

    # Two half-token AllToAll stages (buf0 = dest cols 0:512, buf1 = 512:1024).
    a2a_in = [nc.dram_tensor(f"a2a_in{h}", [NCORES, P, 512], BF16) for h in (0, 1)]
    a2a_out = [nc.dram_tensor(f"a2a_out{h}", [NCORES, P, 512], BF16) for h in (0, 1)]
    # rowsum/reciprocal bounces for the partition-broadcast DMA
    rs_d = nc.dram_tensor("rs_d", [B * NQW, 1024], F32)
    rcp_d = nc.dram_tensor("rcp_d", [B * NQW, 1024], F32)

    with tile.TileContext(nc, pool_alloc_mode="queue") as tc:
        with ExitStack() as ctx:
            const = ctx.enter_context(tc.tile_pool(name="const", bufs=1))
            persist = ctx.enter_context(tc.tile_pool(name="persist", bufs=1))
            xin = ctx.enter_context(tc.tile_pool(name="xin", bufs=3))
            work = ctx.enter_context(tc.tile_pool(name="work", bufs=2))
            expool = ctx.enter_context(tc.tile_pool(name="expool", bufs=4))
            npool = ctx.enter_context(tc.tile_pool(name="npool", bufs=2))
            psum = ctx.enter_context(tc.tile_pool(name="psum", bufs=2, space="PSUM"))

            ident = const.tile([P, P], BF16)
            make_identity(nc, ident)

            # Persistent SBUF: Qt/Kt [hd2, tokens], V-natural-with-ones, w_o.
            qt_sb = persist.tile([P, T], BF16, tag="qt_sb")
            kt_sb = persist.tile([P, T], BF16, tag="kt_sb")
            v_all = persist.tile([P, B * NKT * VROW], BF16, tag="v_all")
            wo_sb = persist.tile([P, NCH * 1024], BF16, tag="wo_sb")
            wq_sb = persist.tile([P, NCH * P], BF16, tag="wq_sb")
            wk_sb = persist.tile([P, NCH * P], BF16, tag="wk_sb")
            wv_sb = persist.tile([P, NCH * P], BF16, tag="wv_sb")

            # ones columns of v_all (cols 64 and 129 of each 130-block)
            v_view = v_all[:].rearrange("p (n c) -> p n c", c=VROW)
            nc.vector.memset(v_view[:, :, HD], 1.0)
            nc.vector.memset(v_view[:, :, 2 * HD + 1], 1.0)

            # weights: per-chunk DMAs on queues that are idle at start
            for ch in range(NCH):
                nc.scalar.dma_start(wk_sb[:, ch * P:(ch + 1) * P], wk_d.ap()[ch])
                nc.gpsimd.dma_start(wv_sb[:, ch * P:(ch + 1) * P], wv_d.ap()[ch])
                nc.scalar.dma_start(wq_sb[:, ch * P:(ch + 1) * P], wq_d.ap()[ch])

            # ---- projection task emitters ----
            xt_tiles = {}
            pend_vt = {}
            x_ds = {"q": xq_d, "k": xk_d, "v": xv_d}
            w_sbs = {"q": wq_sb, "k": wk_sb, "v": wv_sb}

            def dma_task(which, t2):
                def run():
                    t = xin.tile([P, NCH * 1024], BF16, tag="xt")
                    xt_tiles[(which, t2)] = t
                    for ch in range(NCH):
                        nc.sync.dma_start(t[:, ch * 1024:(ch + 1) * 1024],
                                          x_ds[which].ap()[t2, ch])
                return run

            def half_task(which, t2, half):
                def run():
                    xt = xt_tiles[(which, t2)]
                    w_sb = w_sbs[which]
                    ps = psum.tile([P, 1024], F32, tag="sc")
                    for ch in range(NCH):
                        nc.tensor.matmul(
                            ps[:, 0:512], w_sb[:, ch * P:(ch + 1) * P],
                            xt[:, ch * 1024 + half * 512: ch * 1024 + (half + 1) * 512],
                            start=(ch == 0), stop=(ch == NCH - 1))
                    col = t2 * 1024 + half * 512
                    if which == "q":
                        nc.vector.tensor_copy(qt_sb[:, col:col + 512], ps[:, 0:512])
                    elif which == "k":
                        nc.vector.tensor_copy(kt_sb[:, col:col + 512], ps[:, 0:512])
                    else:
                        vt = work.tile([P, 512], BF16, tag="vt")
                        nc.vector.tensor_copy(vt[:], ps[:, 0:512])
                        pend_vt[(t2, half)] = vt
                return run

            def vtrans_task(t2, half):
                # PE transpose V^T [dims, tokens] -> V-natural blocks in
                # v_all, writing around the ones columns. Scratch PSUM comes
                # from the sc rotation (held ~1.5us only).
                def run():
                    vt = pend_vt.pop((t2, half))
                    col = t2 * 1024 + half * 512
                    ps = psum.tile([P, 1024], F32, tag="sc")
                    tp4 = ps[:, 512:768].bitcast(BF16)
                    for j in range(4):
                        tp = tp4[:, j * P:(j + 1) * P]
                        nc.tensor.transpose(tp[:], vt[:, j * P:(j + 1) * P],
                                            ident[:])
                        g = col + j * P
                        bb, ktb = g // S, (g % S) // P
                        base = (bb * NKT + ktb) * VROW
                        nc.vector.tensor_copy(
                            v_all[:, base:base + VROW]
                            .rearrange("p (h c) -> p h c", c=HD + 1)[:, :, 0:HD],
                            tp[:].rearrange("p (h c) -> p h c", c=HD))
                return run

            def wo_task():
                for ch in range(NCH):
                    nc.gpsimd.dma_start(wo_sb[:, ch * 1024:(ch + 1) * 1024],
                                        wo_d.ap()[ch])

            def batch_tasks(b):
                a, b2 = 2 * b, 2 * b + 1
                return [
                    dma_task("k", a), dma_task("v", a), dma_task("k", b2),
                    half_task("k", a, 0), half_task("k", a, 1), dma_task("v", b2),
                    half_task("k", b2, 0), half_task("k", b2, 1),
                    half_task("v", a, 0), half_task("v", a, 1), dma_task("q", a),
                    vtrans_task(a, 0), vtrans_task(a, 1),
                    half_task("v", b2, 0), half_task("v", b2, 1), dma_task("q", b2),
                    vtrans_task(b2, 0), vtrans_task(b2, 1),
                    half_task("q", a, 0), half_task("q", a, 1),
                    half_task("q", b2, 0), half_task("q", b2, 1),
                ]

            # ---- attention ----
            def emit_pv(b, kt, ex, pv0, pv1):
                vb = (b * NKT + kt) * VROW
                nc.tensor.matmul(
                    pv0[:], v_all[:, vb:vb + HD + 1], ex[:, 0:512],
                    start=(kt == 0), stop=(kt == NKT - 1))
                nc.tensor.matmul(
                    pv1[:], v_all[:, vb + HD + 1:vb + VROW], ex[:, 512:1024],
                    start=(kt == 0), stop=(kt == NKT - 1))

            def attn_unit(b, qw, fill, fstate):
                buf = qw % 2
                dest = 2 * b + qw // 2
                qcol = b * S + qw * 512
                pv0 = psum.tile([HD + 1, 512], F32, tag="pv0")
                pv1 = psum.tile([HD + 1, 512], F32, tag="pv1")
                exq = []
                for kt in range(NKT):
                    kcol = b * S + kt * P
                    sc = psum.tile([P, 1024], F32, tag="sc")
                    nc.tensor.matmul(
                        sc[:, 0:512],
                        kt_sb[0:HD, kcol:kcol + P],
                        qt_sb[0:HD, qcol:qcol + 512],
                        start=True, stop=True, tile_position=(0, 0))
                    nc.tensor.matmul(
                        sc[:, 512:1024],
                        kt_sb[HD:2 * HD, kcol:kcol + P],
                        qt_sb[HD:2 * HD, qcol:qcol + 512],
                        start=True, stop=True, tile_position=(HD, 0))
                    ex = expool.tile([P, 1024], BF16, tag="ex")
                    nc.scalar.activation(
                        ex[:], sc[:], mybir.ActivationFunctionType.Exp)
                    exq.append((kt, ex))
                    # filler: keep the PE ahead of the scalar engine
                    fstate[0] += 1
                    target = (len(fill) * fstate[0] + fstate[1] - 1) // fstate[1]
                    while fstate[2] < min(target, len(fill)):
                        fill[fstate[2]]()
                        fstate[2] += 1
                    if len(exq) > 1:
                        k0, e0 = exq.pop(0)
                        emit_pv(b, k0, e0, pv0, pv1)
                for k0, e0 in exq:
                    emit_pv(b, k0, e0, pv0, pv1)
                # normalize. Copy pv off PSUM first (frees the accumulators
                # fast); then the reciprocal bounce runs entirely off-SBUF:
                # ones-rows -> DRAM -> [128,8] reshape, cheap recip, -> DRAM
                # -> partition-broadcast DMAs. Triggers split across the
                # gpsimd and sync queues so per-unit waits don't pile up on
                # one queue.
                unit = b * NQW + qw
                pva = npool.tile([HD + 1, 512], F32, tag="pva")
                pvb = npool.tile([HD + 1, 512], F32, tag="pvb")
                nc.vector.tensor_copy(pva[:], pv0[:])
                nc.vector.tensor_copy(pvb[:], pv1[:])
                nc.sync.dma_start(
                    rs_d.ap()[unit, 0:512].rearrange("(a f) -> a f", a=1),
                    pva[HD:HD + 1, :])
                nc.sync.dma_start(
                    rs_d.ap()[unit, 512:1024].rearrange("(a f) -> a f", a=1),
                    pvb[HD:HD + 1, :])
                rsw = npool.tile([P, 8], F32, tag="rsw")
                nc.sync.dma_start(
                    rsw[:], rs_d.ap()[unit].rearrange("(p f) -> p f", f=8))
                rcw = npool.tile([P, 8], F32, tag="rcw")
                nc.vector.reciprocal(rcw[:], rsw[:])
                nc.sync.dma_start(
                    rcp_d.ap()[unit].rearrange("(p f) -> p f", f=8), rcw[:])
                bca = npool.tile([HD, 512], F32, tag="bca")
                bcb = npool.tile([HD, 512], F32, tag="bcb")
                rl = rcp_d.ap()[unit].rearrange("(a f) -> a f", a=1)
                nc.sync.dma_start(bca[:], rl[:, 0:512].to_broadcast((HD, 512)))
                nc.sync.dma_start(bcb[:], rl[:, 512:1024].to_broadcast((HD, 512)))
                at = npool.tile([P, 512], BF16, tag="at")
                nc.vector.tensor_mul(at[0:HD, :], pva[0:HD, :], bca[:])
                nc.vector.tensor_mul(at[HD:P, :], pvb[0:HD, :], bcb[:])
                nc.sync.dma_start(a2a_in[buf].ap()[dest], at[:])

            def collective(buf):
                nc.gpsimd.collective_compute(
                    "AllToAll", mybir.AluOpType.bypass,
                    replica_groups=[list(range(NCORES))],
                    ins=[a2a_in[buf].ap()], outs=[a2a_out[buf].ap()],
                )

            # ---- output projection for one half of our token slice ----
            def outproj(buf):
                gsb = persist.tile([P, NCH * 512], BF16, tag=f"gsb{buf}")
                for ch in range(NCH):
                    nc.scalar.dma_start(gsb[:, ch * 512:(ch + 1) * 512],
                                        a2a_out[buf].ap()[ch])
                for t128 in range(4):
                    for dhalf in range(2):
                        po = psum.tile([P, 512], F32, tag=("pv0", "pv1")[dhalf])
                        for ch in range(NCH):
                            nc.tensor.matmul(
                                po[:, 0:512],
                                gsb[:, ch * 512 + t128 * P: ch * 512 + (t128 + 1) * P],
                                wo_sb[:, ch * 1024 + dhalf * 512: ch * 1024 + (dhalf + 1) * 512],
                                start=(ch == 0), stop=(ch == NCH - 1))
                        osb = work.tile([P, 512], F32, tag="osb")
                        nc.vector.tensor_copy(osb[:], po[:])
                        row = buf * 512 + t128 * P
                        nc.scalar.dma_start(
                            out_d.ap()[row:row + P, dhalf * 512:(dhalf + 1) * 512],
                            osb[:])

            # ---- schedule ----
            # prologue: only what attn(b0,qw0) needs for its first 8
            # key-tiles — K/V/Q of token-tile 0. Everything else (second
            # token tile, other q-halves) streams in as early fillers,
            # paced densely (over 48 of b0's 64 kt).
            t0 = batch_tasks(0)
            for i in (0, 1, 3, 4, 8, 9, 10, 11, 12, 18):
                t0[i]()
            fill_b0_extra = [t0[i] for i in
                             (2, 5, 6, 7, 13, 16, 14, 17, 19, 15, 20, 21)]

            # b3's attention has no next batch to project; defer a few of its
            # own late-needed tasks (vtranses for its second token tile + 3
            # q-halves) into b3's first unit so the PE stays ahead of exp.
            # b3 runs qw order (1,3,0,2): the cc1 contributors finish two
            # units early so the AllToAlls overlap compute and outproj.
            t3 = batch_tasks(3)
            defer = (16, 17, 21, 18, 20)
            fills = [
                fill_b0_extra + batch_tasks(1),
                batch_tasks(2) + [wo_task],
                [t for i, t in enumerate(t3) if i not in defer],
                [t3[i] for i in defer],
            ]
            for b in range(B):
                fill = fills[b]
                # b3's deferred fillers must all land in its first unit;
                # b0's second token-tile must stream in dense (see above)
                fstate = [0, NKT if b == B - 1 else (48 if b == 0 else NQW * NKT), 0]
                qws = (1, 3, 0, 2) if b == B - 1 else (0, 2, 1, 3)
                for qw in qws:
                    attn_unit(b, qw, fill, fstate)
                    if b == B - 1 and qw == 3:
                        # only cc sits on the gpsimd queue; it fires the
                        # moment (3,1)/(3,3)'s a2a writes land
                        collective(1)
                while fstate[2] < len(fill):
                    fill[fstate[2]]()
                    fstate[2] += 1
            # PE: outproj(1) runs while (3,2)'s recip chain drains and cc0
            # flies; outproj(0) follows once cc0 lands.
            outproj(1)
            collective(0)
            outproj(0)

    if split:
        split_multi_waits(nc)
        _CACHED_NC = nc
    return nc


def _host_prep(query, key, value, w_q, w_k, w_v, w_o):
    sc = 1.0 / np.sqrt(np.float32(HD))

    def tile_x(x):  # [B,S,D] -> [NT2, NCH, 128, 1024] bf16 of x^T
        xt = np.asarray(x, np.float32).reshape(T, D).T          # [D, T]
        xt = xt.reshape(NCH, P, NT2, 1024).transpose(2, 0, 1, 3)
        return np.ascontiguousarray(xt.astype(bf16))

    xq, xk, xv = tile_x(query), tile_x(key), tile_x(value)

    def tile_w(w, c, scale=1.0):  # rows for core c, transposed, chunked
        wc = (np.asarray(w, np.float32)[P * c:P * (c + 1), :] * scale).T  # [D,128]
        return np.ascontiguousarray(wc.reshape(NCH, P, P).astype(bf16))

    wo_t = np.ascontiguousarray(
        np.asarray(w_o, np.float32).T.reshape(NCH, P, 1024).astype(bf16))

    in_maps = []
    for c in range(NCORES):
        in_maps.append({
            "xq": xq, "xk": xk, "xv": xv,
            "wq": tile_w(w_q, c, sc), "wk": tile_w(w_k, c),
            "wv": tile_w(w_v, c), "wo": wo_t,
        })
    return in_maps


def _numpy_fallback(query, key, value, attn_mask, key_padding_mask,
                    w_q, b_q, w_k, b_k, w_v, b_v, w_o, b_o):
    q = query.reshape(T, D) @ w_q.T + b_q
    k = key.reshape(T, D) @ w_k.T + b_k
    v = value.reshape(T, D) @ w_v.T + b_v
    qh = q.reshape(B, S, H, HD).transpose(0, 2, 1, 3)
    kh = k.reshape(B, S, H, HD).transpose(0, 2, 1, 3)
    vh = v.reshape(B, S, H, HD).transpose(0, 2, 1, 3)
    out = np.empty((B, H, S, HD), np.float32)
    neg = np.finfo(np.float32).min
    for b in range(B):
        for h in range(H):
            s = (qh[b, h] @ kh[b, h].T) / np.sqrt(np.float32(HD))
            s = np.where(attn_mask, s, neg)
            s = np.where(key_padding_mask[b][None, :], s, neg)
            s = s - s.max(axis=-1, keepdims=True)
            e = np.exp(s)
            a = e / e.sum(axis=-1, keepdims=True)
            out[b, h] = a @ vh[b, h]
    o = out.transpose(0, 2, 1, 3).reshape(T, D)
    return (o @ w_o.T + b_o).reshape(B, S, D).astype(np.float32)


def kernel(query, key, value, attn_mask, key_padding_mask,
           w_q, b_q, w_k, b_k, w_v, b_v, w_o, b_o):
    query = np.asarray(query, np.float32)
    key = np.asarray(key, np.float32)
    value = np.asarray(value, np.float32)
    attn_mask = np.asarray(attn_mask)
    key_padding_mask = np.asarray(key_padding_mask)
    w_q, b_q = np.asarray(w_q, np.float32), np.asarray(b_q, np.float32)
    w_k, b_k = np.asarray(w_k, np.float32), np.asarray(b_k, np.float32)
    w_v, b_v = np.asarray(w_v, np.float32), np.asarray(b_v, np.float32)
    w_o, b_o = np.asarray(w_o, np.float32), np.asarray(b_o, np.float32)

    if (not attn_mask.all() or not key_padding_mask.all()
            or b_q.any() or b_k.any()):
        return _numpy_fallback(query, key, value, attn_mask, key_padding_mask,
                               w_q, b_q, w_k, b_k, w_v, b_v, w_o, b_o)

    nc = build()
    in_maps = _host_prep(query, key, value, w_q, w_k, w_v, w_o)
    res = run_bass_kernel_spmd(nc, in_maps, list(range(NCORES)))

    out = np.empty((T, D), np.float32)
    for c in range(NCORES):
        out[TOK_PER_CORE * c:TOK_PER_CORE * (c + 1)] = \
            res.results[c]["out"].reshape(TOK_PER_CORE, D)
    # exact host-side bias fold: softmax rows sum to 1 => + (b_v @ w_o.T + b_o)
    out += b_v @ w_o.T + b_o
    return out.reshape(B, S, D)


# revision 43
# speedup vs baseline: 1.2885x; 1.0667x over previous
"""Multi-head attention (B=4, S=2048, D=1024, H=16) on 8 trn2 NeuronCores.

Sharding: tensor-parallel over heads — core c owns heads (2c, 2c+1).

v2 schedule: the attention inner loop is PE-paced. Projection work is
chopped into small tasks (one 8-matmul half = ~1.7us of PE) interleaved
between attention key-tiles so the tensor engine always has more work per
key-tile (~1.17us) than the scalar engine's exp (~1.03us) — the PE never
stalls on exp and stays at max p-state. PV matmuls lag scores by one
key-tile for the same reason.

Per core:
  1. Q^T/K^T projections into [head_dim-stacked, tokens] bf16; V^T
     projection, then DMA-XBAR transposes into V-natural [tokens, head_dim]
     with fused ones-columns (softmax row sums accumulate on PSUM row 64).
  2. Attention per (batch, q-window): scores^T = K^T.T @ Q^T (2-head
     row-packed matmuls), exp on ScalarE (PSUM -> SBUF bf16), PV with
     lhsT=[V|1]. Normalize via single-lane reciprocal + gpsimd
     partition_broadcast (no DRAM bounce).
  3. AllToAll (heads-sharded -> token-sharded), then the output projection
     for this core's 1024-token slice.

Host side folds the 1/sqrt(head_dim) scale into w_q, pre-transposes and
pre-tiles all operands to bf16, and assembles the [4,2048,1024] fp32 output.

Biases are applied exactly on the host: b_v and b_o contribute
(b_v @ w_o.T + b_o) to every token (softmax rows sum to 1). b_q/b_k cannot
be folded; setup_inputs() generates them as zeros — a numpy fallback guards
the (never-exercised) nonzero case, as well as non-trivial masks.
"""

import numpy as np
import ml_dtypes

import concourse.bass as bass
import concourse.tile as tile
from concourse import mybir
from concourse.bass_utils import run_bass_kernel_spmd
from concourse.masks import make_identity

NCORES = 8
B, S, D, H = 4, 2048, 1024, 16
HD = D // H            # 64
P = 128
T = B * S              # 8192 tokens
TOK_PER_CORE = T // NCORES   # 1024
NCH = D // P           # 8 contraction chunks
NT2 = T // 1024        # 8 token tiles of 1024 for projections
NKT = S // P           # 16 key tiles per batch
NQW = S // 512         # 4 q-windows of 512 per batch
VROW = 2 * (HD + 1)    # 130 cols per k-tile in v_all ([V_h0|1|V_h1|1])

BF16 = mybir.dt.bfloat16
F32 = mybir.dt.float32
bf16 = ml_dtypes.bfloat16

_CACHED_NC = None


def split_multi_waits(nc):
    """This walrus build supports one sync-wait per instruction; hoist extras
    onto same-engine NoOps inserted immediately before."""
    for f in nc.m.functions:
        for blk in f.blocks:
            insts = blk.instructions
            i = 0
            while i < len(insts):
                inst = insts[i]
                si = getattr(inst, "sync_info", None)
                if si is not None and si.on_wait and len(si.on_wait) > 1:
                    waits = list(si.on_wait)
                    for j, w in enumerate(waits[:-1]):
                        nop = mybir.InstNoOp(name=f"I-ws-{inst.name}-{j}",
                                             ins=[], outs=[])
                        nop.engine = inst.engine
                        nop.sync_info = mybir.SyncInfo(on_wait=[w], on_update=[])
                        insts.insert(i, nop)
                        i += 1
                    inst.sync_info = mybir.SyncInfo(on_wait=[waits[-1]],
                                                    on_update=si.on_update)
                i += 1


def build(split=True):
    global _CACHED_NC
    if split and _CACHED_NC is not None:
        return _CACHED_NC
    from contextlib import ExitStack

    nc = bass.Bass(num_devices=NCORES, target_bir_lowering=False, debug=False)

    # Inputs (per core). x* are the full activations, tiled on host to
    # [toktile, chunk, 128, 1024].
    xq_d = nc.dram_tensor("xq", [NT2, NCH, P, 1024], BF16, kind="ExternalInput")
    xk_d = nc.dram_tensor("xk", [NT2, NCH, P, 1024], BF16, kind="ExternalInput")
    xv_d = nc.dram_tensor("xv", [NT2, NCH, P, 1024], BF16, kind="ExternalInput")
    wq_d = nc.dram_tensor("wq", [NCH, P, P], BF16, kind="ExternalInput")
    wk_d = nc.dram_tensor("wk", [NCH, P, P], BF16, kind="ExternalInput")
    wv_d = nc.dram_tensor("wv", [NCH, P, P], BF16, kind="ExternalInput")
    wo_d = nc.dram_tensor("wo", [NCH, P, 1024], BF16, kind="ExternalInput")
    out_d = nc.dram_tensor("out", [TOK_PER_CORE, D], F32, kind="ExternalOutput")

    # Two half-token AllToAll stages (buf0 = dest cols 0:512, buf1 = 512:1024).
    a2a_in = [nc.dram_tensor(f"a2a_in{h}", [NCORES, P, 512], BF16) for h in (0, 1)]
    a2a_out = [nc.dram_tensor(f"a2a_out{h}", [NCORES, P, 512], BF16) for h in (0, 1)]
    # rowsum/reciprocal bounces for the partition-broadcast DMA
    rs_d = nc.dram_tensor("rs_d", [B * NQW, 1024], F32)
    rcp_d = nc.dram_tensor("rcp_d", [B * NQW, 1024], F32)
    # tiny warmup AllToAll: absorbs the cc-stream's ~11.5us first-op setup
    ccw_in = nc.dram_tensor("ccw_in", [NCORES, 1, 2], BF16)
    ccw_out = nc.dram_tensor("ccw_out", [NCORES, 1, 2], BF16)

    with tile.TileContext(nc, pool_alloc_mode="queue") as tc:
        with ExitStack() as ctx:
            const = ctx.enter_context(tc.tile_pool(name="const", bufs=1))
            persist = ctx.enter_context(tc.tile_pool(name="persist", bufs=1))
            xin = ctx.enter_context(tc.tile_pool(name="xin", bufs=3))
            work = ctx.enter_context(tc.tile_pool(name="work", bufs=2))
            expool = ctx.enter_context(tc.tile_pool(name="expool", bufs=4))
            npool = ctx.enter_context(tc.tile_pool(name="npool", bufs=2))
            psum = ctx.enter_context(tc.tile_pool(name="psum", bufs=2, space="PSUM"))

            ident = const.tile([P, P], BF16)
            make_identity(nc, ident)

            # Persistent SBUF: Qt/Kt [hd2, tokens], V-natural-with-ones, w_o.
            qt_sb = persist.tile([P, T], BF16, tag="qt_sb")
            kt_sb = persist.tile([P, T], BF16, tag="kt_sb")
            v_all = persist.tile([P, B * NKT * VROW], BF16, tag="v_all")
            wo_sb = persist.tile([P, NCH * 1024], BF16, tag="wo_sb")
            wq_sb = persist.tile([P, NCH * P], BF16, tag="wq_sb")
            wk_sb = persist.tile([P, NCH * P], BF16, tag="wk_sb")
            wv_sb = persist.tile([P, NCH * P], BF16, tag="wv_sb")

            # ones columns of v_all (cols 64 and 129 of each 130-block)
            v_view = v_all[:].rearrange("p (n c) -> p n c", c=VROW)
            nc.vector.memset(v_view[:, :, HD], 1.0)
            nc.vector.memset(v_view[:, :, 2 * HD + 1], 1.0)

            # weights: per-chunk DMAs on queues that are idle at start
            for ch in range(NCH):
                nc.scalar.dma_start(wk_sb[:, ch * P:(ch + 1) * P], wk_d.ap()[ch])
                nc.gpsimd.dma_start(wv_sb[:, ch * P:(ch + 1) * P], wv_d.ap()[ch])
                nc.scalar.dma_start(wq_sb[:, ch * P:(ch + 1) * P], wq_d.ap()[ch])

            # ---- projection task emitters ----
            xt_tiles = {}
            pend_vt = {}
            x_ds = {"q": xq_d, "k": xk_d, "v": xv_d}
            w_sbs = {"q": wq_sb, "k": wk_sb, "v": wv_sb}

            def dma_task(which, t2):
                def run():
                    t = xin.tile([P, NCH * 1024], BF16, tag="xt")
                    xt_tiles[(which, t2)] = t
                    for ch in range(NCH):
                        nc.sync.dma_start(t[:, ch * 1024:(ch + 1) * 1024],
                                          x_ds[which].ap()[t2, ch])
                return run

            # proj bursts and vtranses use the pv tags (idle mid-unit) so the
            # sc rotation serves scores exclusively
            pvtag = [0]

            def next_pvtag():
                pvtag[0] ^= 1
                return ("pv0", "pv1")[pvtag[0]]

            def half_task(which, t2, half):
                def run():
                    xt = xt_tiles[(which, t2)]
                    w_sb = w_sbs[which]
                    ps = psum.tile([P, 512], F32, tag=next_pvtag())
                    for ch in range(NCH):
                        nc.tensor.matmul(
                            ps[:, 0:512], w_sb[:, ch * P:(ch + 1) * P],
                            xt[:, ch * 1024 + half * 512: ch * 1024 + (half + 1) * 512],
                            start=(ch == 0), stop=(ch == NCH - 1))
                    col = t2 * 1024 + half * 512
                    if which == "q":
                        nc.vector.tensor_copy(qt_sb[:, col:col + 512], ps[:, 0:512])
                    elif which == "k":
                        nc.vector.tensor_copy(kt_sb[:, col:col + 512], ps[:, 0:512])
                    else:
                        vt = work.tile([P, 512], BF16, tag="vt")
                        nc.vector.tensor_copy(vt[:], ps[:, 0:512])
                        pend_vt[(t2, half)] = vt
                return run

            def vtrans_task(t2, half):
                # PE transpose V^T [dims, tokens] -> V-natural blocks in
                # v_all, writing around the ones columns. Scratch PSUM comes
                # from the sc rotation (held ~1.5us only).
                def run():
                    vt = pend_vt.pop((t2, half))
                    col = t2 * 1024 + half * 512
                    ps = psum.tile([P, 256], F32, tag=next_pvtag())
                    tp4 = ps[:, 0:256].bitcast(BF16)
                    for j in range(4):
                        tp = tp4[:, j * P:(j + 1) * P]
                        nc.tensor.transpose(tp[:], vt[:, j * P:(j + 1) * P],
                                            ident[:])
                        g = col + j * P
                        bb, ktb = g // S, (g % S) // P
                        base = (bb * NKT + ktb) * VROW
                        nc.vector.tensor_copy(
                            v_all[:, base:base + VROW]
                            .rearrange("p (h c) -> p h c", c=HD + 1)[:, :, 0:HD],
                            tp[:].rearrange("p (h c) -> p h c", c=HD))
                return run

            def wo_task():
                for ch in range(NCH):
                    nc.gpsimd.dma_start(wo_sb[:, ch * 1024:(ch + 1) * 1024],
                                        wo_d.ap()[ch])

            def batch_tasks(b):
                a, b2 = 2 * b, 2 * b + 1
                return [
                    dma_task("k", a), dma_task("v", a), dma_task("k", b2),
                    half_task("k", a, 0), half_task("k", a, 1), dma_task("v", b2),
                    half_task("k", b2, 0), half_task("k", b2, 1),
                    half_task("v", a, 0), half_task("v", a, 1), dma_task("q", a),
                    vtrans_task(a, 0), vtrans_task(a, 1),
                    half_task("v", b2, 0), half_task("v", b2, 1), dma_task("q", b2),
                    vtrans_task(b2, 0), vtrans_task(b2, 1),
                    half_task("q", a, 0), half_task("q", a, 1),
                    half_task("q", b2, 0), half_task("q", b2, 1),
                ]

            # ---- attention ----
            def emit_pv(b, kt, ex, pv0, pv1):
                vb = (b * NKT + kt) * VROW
                nc.tensor.matmul(
                    pv0[:], v_all[:, vb:vb + HD + 1], ex[:, 0:512],
                    start=(kt == 0), stop=(kt == NKT - 1))
                nc.tensor.matmul(
                    pv1[:], v_all[:, vb + HD + 1:vb + VROW], ex[:, 512:1024],
                    start=(kt == 0), stop=(kt == NKT - 1))

            def attn_unit(b, qw, fill, fstate):
                buf = qw % 2
                dest = 2 * b + qw // 2
                qcol = b * S + qw * 512
                pv0 = psum.tile([HD + 1, 512], F32, tag="pv0")
                pv1 = psum.tile([HD + 1, 512], F32, tag="pv1")
                exq = []
                for kt in range(NKT):
                    kcol = b * S + kt * P
                    sc = psum.tile([P, 1024], F32, tag="sc")
                    nc.tensor.matmul(
                        sc[:, 0:512],
                        kt_sb[0:HD, kcol:kcol + P],
                        qt_sb[0:HD, qcol:qcol + 512],
                        start=True, stop=True, tile_position=(0, 0))
                    nc.tensor.matmul(
                        sc[:, 512:1024],
                        kt_sb[HD:2 * HD, kcol:kcol + P],
                        qt_sb[HD:2 * HD, qcol:qcol + 512],
                        start=True, stop=True, tile_position=(HD, 0))
                    ex = expool.tile([P, 1024], BF16, tag="ex")
                    nc.scalar.activation(
                        ex[:], sc[:], mybir.ActivationFunctionType.Exp)
                    exq.append((kt, ex))
                    # filler: keep the PE ahead of the scalar engine
                    fstate[0] += 1
                    target = (len(fill) * fstate[0] + fstate[1] - 1) // fstate[1]
                    while fstate[2] < min(target, len(fill)):
                        fill[fstate[2]]()
                        fstate[2] += 1
                    if len(exq) > 1:
                        k0, e0 = exq.pop(0)
                        emit_pv(b, k0, e0, pv0, pv1)
                for k0, e0 in exq:
                    emit_pv(b, k0, e0, pv0, pv1)
                # normalize. Copy pv off PSUM first (frees the accumulators
                # fast); then the reciprocal bounce runs entirely off-SBUF:
                # ones-rows -> DRAM -> [128,8] reshape, cheap recip, -> DRAM
                # -> partition-broadcast DMAs. Triggers split across the
                # gpsimd and sync queues so per-unit waits don't pile up on
                # one queue.
                unit = b * NQW + qw
                pva = npool.tile([HD + 1, 512], F32, tag="pva")
                pvb = npool.tile([HD + 1, 512], F32, tag="pvb")
                nc.vector.tensor_copy(pva[:], pv0[:])
                nc.vector.tensor_copy(pvb[:], pv1[:])
                nc.sync.dma_start(
                    rs_d.ap()[unit, 0:512].rearrange("(a f) -> a f", a=1),
                    pva[HD:HD + 1, :])
                nc.sync.dma_start(
                    rs_d.ap()[unit, 512:1024].rearrange("(a f) -> a f", a=1),
                    pvb[HD:HD + 1, :])
                rsw = npool.tile([P, 8], F32, tag="rsw")
                nc.sync.dma_start(
                    rsw[:], rs_d.ap()[unit].rearrange("(p f) -> p f", f=8))
                rcw = npool.tile([P, 8], F32, tag="rcw")
                nc.vector.reciprocal(rcw[:], rsw[:])
                nc.sync.dma_start(
                    rcp_d.ap()[unit].rearrange("(p f) -> p f", f=8), rcw[:])
                bca = npool.tile([HD, 512], F32, tag="bca")
                bcb = npool.tile([HD, 512], F32, tag="bcb")
                rl = rcp_d.ap()[unit].rearrange("(a f) -> a f", a=1)
                nc.sync.dma_start(bca[:], rl[:, 0:512].to_broadcast((HD, 512)))
                nc.sync.dma_start(bcb[:], rl[:, 512:1024].to_broadcast((HD, 512)))
                at = npool.tile([P, 512], BF16, tag="at")
                nc.vector.tensor_mul(at[0:HD, :], pva[0:HD, :], bca[:])
                nc.vector.tensor_mul(at[HD:P, :], pvb[0:HD, :], bcb[:])
                nc.sync.dma_start(a2a_in[buf].ap()[dest], at[:])

            def collective(buf):
                nc.gpsimd.collective_compute(
                    "AllToAll", mybir.AluOpType.bypass,
                    replica_groups=[list(range(NCORES))],
                    ins=[a2a_in[buf].ap()], outs=[a2a_out[buf].ap()],
                )

            # ---- output projection for one half of our token slice ----
            def outproj(buf):
                gsb = persist.tile([P, NCH * 512], BF16, tag=f"gsb{buf}")
                for ch in range(NCH):
                    nc.scalar.dma_start(gsb[:, ch * 512:(ch + 1) * 512],
                                        a2a_out[buf].ap()[ch])
                for t128 in range(4):
                    for dhalf in range(2):
                        po = psum.tile([P, 512], F32, tag=("pv0", "pv1")[dhalf])
                        for ch in range(NCH):
                            nc.tensor.matmul(
                                po[:, 0:512],
                                gsb[:, ch * 512 + t128 * P: ch * 512 + (t128 + 1) * P],
                                wo_sb[:, ch * 1024 + dhalf * 512: ch * 1024 + (dhalf + 1) * 512],
                                start=(ch == 0), stop=(ch == NCH - 1))
                        osb = work.tile([P, 512], F32, tag="osb")
                        nc.vector.tensor_copy(osb[:], po[:])
                        row = buf * 512 + t128 * P
                        nc.scalar.dma_start(
                            out_d.ap()[row:row + P, dhalf * 512:(dhalf + 1) * 512],
                            osb[:])

            # ---- schedule ----
            # prologue: only what attn(b0,qw0) needs for its first 8
            # key-tiles — K/V/Q of token-tile 0. Everything else (second
            # token tile, other q-halves) streams in as early fillers,
            # paced densely (over 48 of b0's 64 kt).
            t0 = batch_tasks(0)
            for i in (0, 1, 3, 4, 8, 9, 10, 11, 12, 18):
                t0[i]()
            # warm the collective stream while early attention runs
            nc.gpsimd.collective_compute(
                "AllToAll", mybir.AluOpType.bypass,
                replica_groups=[list(range(NCORES))],
                ins=[ccw_in.ap()], outs=[ccw_out.ap()],
            )
            fill_b0_extra = [t0[i] for i in
                             (2, 5, 6, 7, 13, 16, 14, 17, 19, 15, 20, 21)]

            # b3's attention has no next batch to project; defer a few of its
            # own late-needed tasks (vtranses for its second token tile + 3
            # q-halves) into b3's first unit so the PE stays ahead of exp.
            # b3 runs qw order (1,3,0,2): the cc1 contributors finish two
            # units early so the AllToAlls overlap compute and outproj.
            t3 = batch_tasks(3)
            defer = (16, 17, 21, 18, 20)
            fills = [
                fill_b0_extra + batch_tasks(1),
                batch_tasks(2) + [wo_task],
                [t for i, t in enumerate(t3) if i not in defer],
                [t3[i] for i in defer],
            ]
            for b in range(B):
                fill = fills[b]
                # b3's deferred fillers must all land in its first unit;
                # b0's second token-tile must stream in dense (see above)
                fstate = [0, NKT if b == B - 1 else (48 if b == 0 else NQW * NKT), 0]
                qws = (1, 3, 0, 2) if b == B - 1 else (0, 2, 1, 3)
                for qw in qws:
                    attn_unit(b, qw, fill, fstate)
                    if b == B - 1 and qw == 3:
                        # only cc sits on the gpsimd queue; it fires the
                        # moment (3,1)/(3,3)'s a2a writes land
                        collective(1)
                while fstate[2] < len(fill):
                    fill[fstate[2]]()
                    fstate[2] += 1
            # PE: outproj(1) runs while (3,2)'s recip chain drains and cc0
            # flies; outproj(0) follows once cc0 lands.
            outproj(1)
            collective(0)
            outproj(0)

    if split:
        split_multi_waits(nc)
        _CACHED_NC = nc
    return nc


def _host_prep(query, key, value, w_q, w_k, w_v, w_o):
    sc = 1.0 / np.sqrt(np.float32(HD))

    def tile_x(x):  # [B,S,D] -> [NT2, NCH, 128, 1024] bf16 of x^T
        xt = np.asarray(x, np.float32).reshape(T, D).T          # [D, T]
        xt = xt.reshape(NCH, P, NT2, 1024).transpose(2, 0, 1, 3)
        return np.ascontiguousarray(xt.astype(bf16))

    xq, xk, xv = tile_x(query), tile_x(key), tile_x(value)

    def tile_w(w, c, scale=1.0):  # rows for core c, transposed, chunked
        wc = (np.asarray(w, np.float32)[P * c:P * (c + 1), :] * scale).T  # [D,128]
        return np.ascontiguousarray(wc.reshape(NCH, P, P).astype(bf16))

    wo_t = np.ascontiguousarray(
        np.asarray(w_o, np.float32).T.reshape(NCH, P, 1024).astype(bf16))

    in_maps = []
    for c in range(NCORES):
        in_maps.append({
            "xq": xq, "xk": xk, "xv": xv,
            "wq": tile_w(w_q, c, sc), "wk": tile_w(w_k, c),
            "wv": tile_w(w_v, c), "wo": wo_t,
        })
    return in_maps


def _numpy_fallback(query, key, value, attn_mask, key_padding_mask,
                    w_q, b_q, w_k, b_k, w_v, b_v, w_o, b_o):
    q = query.reshape(T, D) @ w_q.T + b_q
    k = key.reshape(T, D) @ w_k.T + b_k
    v = value.reshape(T, D) @ w_v.T + b_v
    qh = q.reshape(B, S, H, HD).transpose(0, 2, 1, 3)
    kh = k.reshape(B, S, H, HD).transpose(0, 2, 1, 3)
    vh = v.reshape(B, S, H, HD).transpose(0, 2, 1, 3)
    out = np.empty((B, H, S, HD), np.float32)
    neg = np.finfo(np.float32).min
    for b in range(B):
        for h in range(H):
            s = (qh[b, h] @ kh[b, h].T) / np.sqrt(np.float32(HD))
            s = np.where(attn_mask, s, neg)
            s = np.where(key_padding_mask[b][None, :], s, neg)
            s = s - s.max(axis=-1, keepdims=True)
            e = np.exp(s)
            a = e / e.sum(axis=-1, keepdims=True)
            out[b, h] = a @ vh[b, h]
    o = out.transpose(0, 2, 1, 3).reshape(T, D)
    return (o @ w_o.T + b_o).reshape(B, S, D).astype(np.float32)


def kernel(query, key, value, attn_mask, key_padding_mask,
           w_q, b_q, w_k, b_k, w_v, b_v, w_o, b_o):
    query = np.asarray(query, np.float32)
    key = np.asarray(key, np.float32)
    value = np.asarray(value, np.float32)
    attn_mask = np.asarray(attn_mask)
    key_padding_mask = np.asarray(key_padding_mask)
    w_q, b_q = np.asarray(w_q, np.float32), np.asarray(b_q, np.float32)
    w_k, b_k = np.asarray(w_k, np.float32), np.asarray(b_k, np.float32)
    w_v, b_v = np.asarray(w_v, np.float32), np.asarray(b_v, np.float32)
    w_o, b_o = np.asarray(w_o, np.float32), np.asarray(b_o, np.float32)

    if (not attn_mask.all() or not key_padding_mask.all()
            or b_q.any() or b_k.any()):
        return _numpy_fallback(query, key, value, attn_mask, key_padding_mask,
                               w_q, b_q, w_k, b_k, w_v, b_v, w_o, b_o)

    nc = build()
    in_maps = _host_prep(query, key, value, w_q, w_k, w_v, w_o)
    res = run_bass_kernel_spmd(nc, in_maps, list(range(NCORES)))

    out = np.empty((T, D), np.float32)
    for c in range(NCORES):
        out[TOK_PER_CORE * c:TOK_PER_CORE * (c + 1)] = \
            res.results[c]["out"].reshape(TOK_PER_CORE, D)
    # exact host-side bias fold: softmax rows sum to 1 => + (b_v @ w_o.T + b_o)
    out += b_v @ w_o.T + b_o
    return out.reshape(B, S, D)
